# revision 60
# baseline (speedup 1.0000x reference)
"""Single-head attention kernel for Trainium2, 8 NeuronCores SPMD (v2).

Problem: x[4,4096,1024] @ {Wq,Wk,Wv}[1024,128] -> q,k,v; softmax(q k^T/sqrt(128)) v.

Sharding: core c -> (batch b = c//2, query-half h = c%2).
Each core receives xT = x[b].T in bf16 with the 4096 columns permuted so
"my" 2048 query rows come first; it computes kT/vn for all 4096 keys, qT
for its 2048 queries, and emits outT [128, 2048] fp32.

Key design points vs the f32r two-phase version (181us -> target ~115us):
  * bf16 everywhere on the matmul paths: halves the x DMA (8MB), halves
    SBUF footprints, enables FWL fast weight loads, and gives 2x DVE on
    the exp-sum (denominator) adds. PSUM stays fp32.
  * ONE fused pass: projections of each 1024-key group are interleaved
    with attention on already-projected chunks (catch-up schedule), so
    PE never idles waiting for a phase barrier and stays HAM-warm.
  * Denominator entirely off the PE: dacc (bf16, DVE) accumulates the
    exp tiles; a single ones-matmul per q-block does the partition sum
    at the end. The old version spent ~27us of PE streaming on it.
  * Bias algebra: bk shifts every score of a query by a constant, which
    softmax cancels -> dropped. bv is added at the output (attn rows sum
    to 1). Only bq is applied (on the qT evacuation).

On-chip layouts (SBUF [128 part x free]):
  kT,qT [d=128, seq] bf16; vn = natural-v chunks [128, S] bf16;
  u = exp tiles [128, 2 x 512*len(pair)] bf16 (two exps share one tile
  so the DVE denominator add runs once per pair of chunks);
  out_acc [128, SQ] fp32 accumulates PV partials evacuated from PSUM.

PSUM (8 banks exactly):
  pps 2x[128,512] f32  - projection accumulators + v-transpose outputs
  sps 2x[128,1024] f32 - score tiles (2 q-blocks wide), 1024-wide exp
  po  1x[128,1024] f32 - PV accumulator for a q-block pair over an
                         8/16-chunk range, then DVE-added into out_acc
                         (also recycled for the dps/bps epilogue tiles)
"""

import math

import numpy as np

import concourse.bacc as bacc
import concourse.bass as bass
import concourse.mybir as mybir
from concourse.bass import ts
from concourse.masks import make_identity
from concourse.tile import TileContext

P = 128
D_MODEL = 1024
D_QK = 128
B = 4
S_FULL = 4096
N_CORES = 8
BLK = 512  # projection block (columns of xT)
BPG = 2  # blocks per group (projection/DMA granularity)

F32 = mybir.dt.float32
F32R = mybir.dt.float32r
BF16 = mybir.dt.bfloat16
AF = mybir.ActivationFunctionType

SM_SCALE = 1.0 / math.sqrt(D_QK)


def build_attention(nc: bass.Bass, S: int = S_FULL, SQ: int = S_FULL // 2, repeat: int = 1):
    """Emit the SPMD single-core program. S = #keys, SQ = #queries."""
    DC = D_MODEL // P  # 8 d_model chunks
    NBLK = S // BLK  # xT column blocks
    QNB = SQ // BLK  # query blocks
    assert NBLK % BPG == 0 and QNB <= NBLK
    NG = NBLK // BPG  # groups
    CPB = BLK // P  # k-chunks per block (4)
    CPG = CPB * BPG  # k-chunks per group (8)
    KC = S // P  # total k chunks
    # q-block pairs; pair p covers q-blocks (2p, 2p+1)
    pairs = [tuple(range(i, min(i + 2, QNB))) for i in range(0, QNB, 2)]
    # group after which pair p's qT columns exist
    avail = [max(pr) // BPG for pr in pairs]

    # All inputs are pre-arranged on the host into the exact SBUF layouts so
    # every DMA is a contiguous 1:1 copy with >=2KB runs per partition.
    # Strided descriptor-per-chunk DMAs were the kernel's real bottleneck:
    # ~11k descriptors made the DMA engines the pacing unit (~177us) while
    # compute finished in ~122us.
    xh = nc.dram_tensor("xh", [NBLK * P, DC * BLK], BF16, kind="ExternalInput").ap()
    wq = nc.dram_tensor("Wq", [P, DC * D_QK], BF16, kind="ExternalInput").ap()
    wk = nc.dram_tensor("Wk", [P, DC * D_QK], BF16, kind="ExternalInput").ap()
    wv = nc.dram_tensor("Wv", [P, DC * D_QK], BF16, kind="ExternalInput").ap()
    bqv = nc.dram_tensor("bqv", [2, D_QK], F32R, kind="ExternalInput").ap()
    outT = nc.dram_tensor("outT", [D_QK, SQ], F32, kind="ExternalOutput").ap()

    with TileContext(nc) as tc:
        lp = nc.allow_low_precision(reason="bf16 accumulate of positive exp values")
        lp.__enter__()
        if repeat > 1:
            loop_cm = tc.For_i(0, repeat, 1)
            loop_cm.__enter__()
        with (
            tc.tile_pool(name="persist", bufs=1) as pp,
            tc.tile_pool(name="xt_pool", bufs=NBLK) as xp,
            tc.tile_pool(name="u_pool", bufs=6) as up,
            tc.tile_pool(name="wkb", bufs=6) as wkb,
            tc.tile_pool(name="pps", bufs=2, space="PSUM") as pps,
            tc.tile_pool(name="sps", bufs=2, space="PSUM") as spsp,
            tc.tile_pool(name="po", bufs=1, space="PSUM") as pop,
        ):
            # --- tiles ---
            w_sb = {
                nm: pp.tile([P, DC * D_QK], BF16, tag=f"w{nm}", name=f"w{nm}_sb")
                for nm in ("q", "k", "v")
            }
            w_dram = {"q": wq, "k": wk, "v": wv}
            bqv_row = pp.tile([2, D_QK], F32R, tag="bqv_row")
            bq_sb = pp.tile([P, 1], F32, tag="bq_sb")
            bv_sb = pp.tile([P, 1], F32, tag="bv_sb")
            ident = pp.tile([P, P], F32, tag="ident")
            make_identity(nc, ident)
            ident_r = pp.tile([P, P], F32R, tag="ident_r")
            nc.vector.tensor_copy(out=ident_r, in_=ident)
            ones_col = pp.tile([P, 1], BF16, tag="ones_col")  # lhsT for partition-sum
            nc.gpsimd.memset(ones_col, 1.0)
            ones_row = pp.tile([1, P], F32, tag="ones_row")
            nc.gpsimd.memset(ones_row, 1.0)
            ones_row_r = pp.tile([1, P], F32R, tag="ones_row_r")  # lhsT for bcast
            nc.vector.tensor_copy(out=ones_row_r, in_=ones_row)

            kT = pp.tile([P, S], BF16, tag="kT")
            vn = pp.tile([P, S], BF16, tag="vn")
            qT = pp.tile([P, SQ], BF16, tag="qT")
            out_acc = pp.tile([P, SQ], F32, tag="out_acc")
            daccs = [
                pp.tile([P, 2 * BLK * len(pr)], BF16, tag=f"dacc{p}", name=f"dacc{p}")
                for p, pr in enumerate(pairs)
            ]

            def dma_w(nm):
                nc.sync.dma_start(out=w_sb[nm], in_=w_dram[nm])

            # Wait-absorbers: LDWEIGHTS can encode only one sync wait; tiny
            # PE matmuls read each DMA'd / off-engine-produced tile so real
            # matmuls carry at most one un-observed semaphore (the rest are
            # split into event semaphores by nc.compile(), which is slow).
            def pe_absorb_simple(ap):
                # wait-absorber: a tiny matmul reads col 0 of the tile so PE
                # observes its producer semaphore; real matmuls then carry at
                # most one un-observed wait (the LDWEIGHTS wait-slot limit)
                scr = pps.tile([P, BLK], F32, tag="pps", name="absorb_scr")
                a = ap[:, 0:1]
                if a.dtype == F32R:
                    a = a.bitcast(F32)
                nc.tensor.matmul(scr[0:1, 0:1], a, a, start=True, stop=True)

            # --- DMA issue order (one queue set, ~in-order): Wk and block 0
            # first so the first projection starts ASAP, then the rest.
            xts = []
            hc = DC // 2

            def dma_x(b):
                xt = xp.tile([P, DC * BLK], BF16, tag="xt", name=f"xt_{b}")
                # two halves so each projection matmul carries ONE dma wait
                src = xh[b * P : (b + 1) * P, :]
                if b < 2:
                    # startup-critical blocks: two halves so the first
                    # projections begin after half a block arrives
                    nc.sync.dma_start(out=xt[:, : hc * BLK], in_=src[:, : hc * BLK])
                    nc.sync.dma_start(out=xt[:, hc * BLK :], in_=src[:, hc * BLK :])
                else:
                    nc.sync.dma_start(out=xt, in_=src)
                xts.append(xt.rearrange("p (c s) -> p c s", s=BLK))

            nc.sync.dma_start(out=bqv_row, in_=bqv)
            dma_w("k")
            dma_x(0)
            dma_w("q")
            dma_x(1)
            dma_w("v")
            for b in range(2, NBLK):
                dma_x(b)

            absorbed = set()

            def absorb_once(key, ap):
                if key not in absorbed:
                    pe_absorb_simple(ap)
                    absorbed.add(key)

            # broadcast bq/bv rows ([2,128] dram) to [128,1] per-partition
            # scalars via one tiny transpose-by-matmul against I2
            absorb_once("bqv", bqv_row)  # bias MM then waits only ident_r
            bias_ps = pop.tile([P, 2 * BLK], F32, tag="po", name="bias_ps")
            nc.tensor.matmul(
                bias_ps[:, 0:2], bqv_row, ident_r[0:2, 0:2], start=True, stop=True
            )
            nc.vector.tensor_copy(out=bq_sb, in_=bias_ps[:, 0:1])
            nc.vector.tensor_copy(out=bv_sb, in_=bias_ps[:, 1:2])

            def proj_k(b, absorb=True):
                absorb_once("wk", w_sb["k"])
                kps = pps.tile([P, BLK], F32, tag="pps", name=f"kps_{b}")
                for c in range(DC):
                    nc.tensor.matmul(
                        kps,
                        w_sb["k"][:, ts(c, D_QK)],
                        xts[b][:, c],
                        start=(c == 0),
                        stop=(c == DC - 1),
                    )
                nc.vector.tensor_copy(out=kT[:, ts(b, BLK)], in_=kps)
                if absorb:
                    pe_absorb_simple(kT[:, ts(b, BLK)])

            def proj_v(b, absorb=True):
                absorb_once("wv", w_sb["v"])
                absorb_once("ident", ident)
                vps = pps.tile([P, BLK], F32, tag="pps", name=f"vps_{b}")
                for c in range(DC):
                    nc.tensor.matmul(
                        vps,
                        w_sb["v"][:, ts(c, D_QK)],
                        xts[b][:, c],
                        start=(c == 0),
                        stop=(c == DC - 1),
                    )
                # bv folds in here for free: dv is the partition dim of vT,
                # and attention rows sum to 1, so out = attn @ (v + bv)
                vt_tmp = wkb.tile([P, BLK], F32R, tag="vt_tmp", name=f"vt_{b}")
                nc.vector.tensor_scalar_add(vt_tmp, vps, bv_sb)
                tps = pps.tile([P, BLK], F32, tag="pps", name=f"tps_{b}")
                for j in range(CPB):
                    nc.tensor.transpose(
                        tps[:, ts(j, P)], vt_tmp[:, ts(j, P)].bitcast(F32), ident
                    )
                nc.vector.tensor_copy(out=vn[:, ts(b, BLK)], in_=tps)
                if absorb:
                    pe_absorb_simple(vn[:, ts(b, BLK)])

            def proj_q(b, absorb=True):
                absorb_once("wq", w_sb["q"])
                qps = pps.tile([P, BLK], F32, tag="pps", name=f"qps_{b}")
                for c in range(DC):
                    nc.tensor.matmul(
                        qps,
                        w_sb["q"][:, ts(c, D_QK)],
                        xts[b][:, c],
                        start=(c == 0),
                        stop=(c == DC - 1),
                    )
                nc.vector.tensor_scalar_add(qT[:, ts(b, BLK)], qps, bq_sb)
                if absorb:
                    pe_absorb_simple(qT[:, ts(b, BLK)])

            def proj_block(b):
                proj_k(b)
                proj_v(b)
                if b < QNB:
                    proj_q(b)

            dacc_started = [False] * len(pairs)
            done_units = [0] * len(pairs)
            total_units = [KC] * len(pairs)
            last_evac = [None]  # most recent po-evac destination (for absorb)

            def epilogue_pair(p, pr):
                w = BLK * len(pr)
                dacc = daccs[p]
                absorb_once("ones_col", ones_col)
                absorb_once("ones_row_r", ones_row_r[0:1, 0:1].broadcast_to([1, 1]))
                fin = wkb.tile([P, w], F32, tag="fin", name=f"fin_{p}")
                for j, qb in enumerate(pr):
                    # one PSUM tile per qb (dps bank 0, bps bank 1); the two
                    # qb chains use different pools so they run in parallel
                    pool = pop if j == 0 else spsp
                    dtile = pool.tile(
                        [P, 2 * BLK], F32,
                        tag="po" if j == 0 else "sps", name=f"ep_{qb}",
                    )
                    dps = dtile[0:1, 0:BLK]
                    # partition-sum of both chunk-parity halves of dacc
                    nc.tensor.matmul(
                        dps, ones_col, dacc[:, ts(j, BLK)], start=True, stop=False
                    )
                    nc.tensor.matmul(
                        dps, ones_col, dacc[:, w + j * BLK : w + (j + 1) * BLK],
                        start=False, stop=True,
                    )
                    rec = wkb.tile([1, BLK], F32R, tag="rec", name=f"rec_{qb}")
                    nc.vector.reciprocal(out=rec, in_=dps)
                    bps = dtile[:, BLK : 2 * BLK]
                    nc.tensor.matmul(bps, ones_row_r, rec, start=True, stop=True)
                    nc.vector.tensor_mul(
                        out=fin[:, ts(j, BLK)], in0=out_acc[:, ts(qb, BLK)], in1=bps
                    )
                    nc.sync.dma_start(
                        out=outT[:, ts(qb, BLK)], in_=fin[:, ts(j, BLK)]
                    )

            def att_group(g, jobs):
                """Attention for group g, with projection `jobs` (thunks)
                interleaved evenly between iterations to keep PE fed while
                ACT chews the exps."""
                # build the iteration stream first
                iters = []  # (p, pr, c, first, last)
                pair_order = (
                    list(enumerate(pairs))[::-1]
                    if g == NG - 1
                    else list(enumerate(pairs))
                )
                for p, pr in pair_order:
                    if g < avail[p]:
                        continue
                    lo = g * CPG if g > avail[p] else 0
                    rng = list(range(lo, (g + 1) * CPG))
                    for i, c in enumerate(rng):
                        iters.append((p, pr, c, i == 0, i == len(rng) - 1))
                job_at = {}
                for pos, job in jobs:
                    job_at.setdefault(pos, []).append(job)
                pos = {}  # p -> (po tile, u tile, parity)
                for it, (p, pr, c, first, last) in enumerate(iters):
                    for job in job_at.get(it, []):
                        job()
                    w = BLK * len(pr)
                    if first:
                        if last_evac[0] is not None:
                            # observe the previous po evac so the first PV of
                            # this range carries only its u wait
                            pe_absorb_simple(last_evac[0])
                            last_evac[0] = None
                        pos[p] = [
                            pop.tile([P, 2 * BLK], F32, tag="po", name=f"po_{g}_{p}"),
                            None,
                        ]
                    po = pos[p][0]
                    sps = spsp.tile([P, 1024], F32, tag="sps", name=f"s_{g}_{p}_{c}")
                    for j, qb in enumerate(pr):
                        nc.tensor.matmul(
                            sps[:, ts(j, BLK)],
                            kT[:, ts(c, P)],
                            qT[:, ts(qb, BLK)],
                            start=True,
                            stop=True,
                        )
                    par = c % 2
                    if par == 0:
                        pos[p][1] = up.tile(
                            [P, 2 * w], BF16, tag="u", name=f"u_{g}_{p}_{c}"
                        )
                    u = pos[p][1]
                    us = u[:, par * w : par * w + w]
                    nc.scalar.activation(us, sps[:, 0:w], AF.Exp, scale=SM_SCALE)

                    def dacc_add():
                        if not dacc_started[p]:
                            nc.vector.tensor_copy(out=daccs[p], in_=u)
                            dacc_started[p] = True
                        else:
                            nc.vector.tensor_add(out=daccs[p], in0=daccs[p], in1=u)

                    if par == 1 and not last:
                        dacc_add()
                    for j, qb in enumerate(pr):
                        nc.tensor.matmul(
                            po[:, ts(j, BLK)],
                            vn[:, ts(c, P)],
                            us[:, ts(j, BLK)],
                            start=first,
                            stop=last,
                        )
                    if last:
                        # evacuate po into out_acc; emitted BEFORE the final
                        # dacc add so the next range's first PV (which WARs
                        # on the po slot) isn't queued behind it on DVE
                        dst = out_acc[:, pr[0] * BLK : (pr[-1] + 1) * BLK]
                        if done_units[p] == 0:
                            nc.vector.tensor_copy(out=dst, in_=po[:, 0:w])
                        else:
                            nc.vector.tensor_add(out=dst, in0=dst, in1=po[:, 0:w])
                        last_evac[0] = dst
                        if par == 1:
                            dacc_add()
                        done_units[p] += CPG if g > avail[p] else (g + 1) * CPG
                        if done_units[p] == total_units[p]:
                            epilogue_pair(p, pr)

            # Emission plan: minimal prologue (kT block 0 + the qT blocks the
            # first pair needs), then every remaining projection is a JIT job
            # spread into the att groups just ahead of its deadline, so
            # per-window PE work tracks ACT's exp load.
            if NG == 4 and QNB == 4:
                proj_k(0)
                proj_q(0)
                proj_q(1)
                J = lambda f, b: (lambda: f(b))  # noqa: E731
                # (iteration, job): every projection must be emitted before
                # the first iteration whose scores/PV reads it (PE is
                # in-order, so a late job would deadlock)
                jobs_by_g = {
                    0: [(0, J(proj_v, 0)), (1, J(proj_k, 1)), (2, J(proj_v, 1)),
                        (3, J(proj_k, 2)), (4, J(proj_v, 2)), (5, J(proj_q, 2)),
                        (6, J(proj_k, 3)), (7, J(proj_v, 3))],
                    1: [(0, J(proj_q, 3)), (2, J(proj_k, 4)), (5, J(proj_v, 4)),
                        (8, J(proj_k, 5)), (11, J(proj_v, 5))],
                    2: [(0, J(proj_k, 6)), (4, J(proj_v, 6))],
                    3: [(0, J(proj_k, 7)), (2, J(proj_v, 7))],
                }
            else:
                for b in range(NBLK):
                    proj_block(b)
                jobs_by_g = {}
            for g in range(NG):
                att_group(g, jobs_by_g.get(g, []))

        if repeat > 1:
            loop_cm.__exit__(None, None, None)

    return nc


_NC_CACHE: dict = {}


def _get_nc(S: int = S_FULL, SQ: int = S_FULL // 2, repeat: int = 1):
    key = (S, SQ, repeat)
    if key not in _NC_CACHE:
        nc = bacc.Bacc("TRN2", debug=False)
        build_attention(nc, S, SQ, repeat)
        nc.compile()  # splits multi-waits into event semaphores (HW limit)
        _NC_CACHE[key] = nc
    return _NC_CACHE[key]


def _w_layout(W, bf16):
    # [D, 128] -> [128 part, DC*128]: w[p, c*128+n] = W[c*128+p, n]
    return np.ascontiguousarray(
        np.asarray(W, np.float32).reshape(8, P, D_QK).transpose(1, 0, 2).reshape(P, -1)
    ).astype(bf16)


def make_in_maps(x, Wq, bq, Wk, bk, Wv, bv):
    """Per-core input dicts. Core c = (batch c//2, query-half c%2).

    All tensors are pre-arranged into the kernel's SBUF layouts so on-device
    DMAs are contiguous (descriptor-count-bound otherwise). bk is
    mathematically irrelevant (it adds a per-query constant to that query's
    scores; softmax is shift-invariant), so it is dropped.
    """
    import ml_dtypes

    bf16 = ml_dtypes.bfloat16
    x = np.asarray(x, dtype=np.float32)
    NBLK = S_FULL // 512
    common = {
        "Wq": _w_layout(Wq, bf16),
        "Wk": _w_layout(Wk, bf16),
        "Wv": _w_layout(Wv, bf16),
        "bqv": np.ascontiguousarray(
            np.stack([np.asarray(bq), np.asarray(bv)]).astype(np.float32)
        ),
    }
    in_maps = []
    for c in range(N_CORES):
        b, h = divmod(c, 2)
        xb = x[b]  # [S, D]
        half = S_FULL // 2
        if h == 0:
            perm = xb
        else:
            perm = np.concatenate([xb[half:], xb[:half]], axis=0)
        # xh[b*128+p, c*512+s] = perm[b*512+s, c*128+p]
        xhf = (
            perm.reshape(NBLK, 512, 8, P)
            .transpose(0, 3, 2, 1)
            .reshape(NBLK * P, 8 * 512)
        )
        in_maps.append({"xh": np.ascontiguousarray(xhf).astype(bf16), **common})
    return in_maps


def assemble_output(results):
    """results: list of 8 per-core dicts with 'outT' [128, 2048]."""
    half = S_FULL // 2
    out = np.empty((B, S_FULL, D_QK), dtype=np.float32)
    for c in range(N_CORES):
        b, h = divmod(c, 2)
        out[b, h * half : (h + 1) * half, :] = results[c]["outT"].T
    return out


def kernel(x, Wq, bq, Wk, bk, Wv, bv):
    from concourse.bass_utils import run_bass_kernel_spmd

    nc = _get_nc()
    in_maps = make_in_maps(x, Wq, bq, Wk, bk, Wv, bv)
    res = run_bass_kernel_spmd(nc, in_maps, list(range(N_CORES)))
    return assemble_output(res.results)
